# revision 10
# baseline (speedup 1.0000x reference)
"""Multi-head attention (no softmax) on 8 trn2 NeuronCores.

Reference: out = ((x @ Wqkv.T -> q,k,v per head) ; (q @ k.T * s) @ v ; concat ; @ Wproj.T)

No softmax -> attention is linear:
    (q @ k.T) @ v == q @ (k.T @ v),  k.T @ v is only 64x64 per head,
so the T x T score matrices never exist. Per head:
    M_h = (s * k_h).T @ v_h        (64 x 64, reduced over the batch's tokens)
    out += (q_h @ M_h) @ Wproj_h.T

Sharding: token-parallel. Core c owns batch b=c//2, token half c%2 (512 tokens).
M_h needs a reduction over the full batch -> two tiny pairwise AllReduces,
fired as early as possible (after each kv half) and consumed as late as
possible (att blocks 0-3 only after the q 0-3 matmuls, att 4-7 after the first
out wave) to ride out the ~15-20us trigger->data collective latency.

All matmul operands are bfloat16 (same 1 cycle/row PE rate as fp32r, half the
HBM traffic; this problem sits at the compute/memory ridge so fp32 DMA was the
pacer). PSUM accumulates fp32; evictions cast to bf16 except the final output.
The head-dim scale 1/8 is folded into W_k on the host (exact).

Layouts (contraction dim on partitions, 2KB DMA lines):
  wkv (E, 2048): [k half0 | v half0 | k half1 | v half1], features grouped
                 h*64+j, k scaled by 1/8
  wq  (E, 1024): q features grouped;  wp (E, 1024): wp[f, o] = W_proj[o, f]
  x2  (128, 4096): e-block pairs packed, x2[p, 512*e + t] = x[b, tok, 128e+p]

Schedule notes (from perfetto traces):
  - each dma_start trigger occupies its sequencer ~0.8us -> spread the input
    stream over the sync/scalar/vector/gpsimd queues instead of sync alone
  - the PE p-state drops to 1.2GHz after any idle gap (3us to re-ramp), so kv
    half A runs k0 x4 + v0 x2 PSUM banks e-outer, giving the PE 6 matmuls per
    arriving (x2, wA) tile pair -- no starvation gaps while the stream lands
  - collective readbacks ride the sync queue (idle after weights); bounces on
    gpsimd; neither blocks an eviction engine
"""

import numpy as np

B, T, E = 4, 1024, 1024
NH, HD = 16, 64
N_CORES = 8
TPC = T // 2  # tokens per core = 512

_built = None


def _build():
    """Build + compile the 8-core SPMD Bass program once."""
    global _built
    if _built is not None:
        return _built

    import concourse.mybir as mybir
    import concourse.tile as tile
    from concourse import bacc

    f32 = mybir.dt.float32
    bf16 = mybir.dt.bfloat16
    GROUPS = [[0, 1], [2, 3], [4, 5], [6, 7]]

    nc = bacc.Bacc("TRN2", target_bir_lowering=False, debug=False, num_devices=N_CORES)
    x2 = nc.dram_tensor("x2", [128, 4096], bf16, kind="ExternalInput").ap()
    wkv = nc.dram_tensor("wkv", [E, 2048], bf16, kind="ExternalInput").ap()
    wq = nc.dram_tensor("wq", [E, E], bf16, kind="ExternalInput").ap()
    wp = nc.dram_tensor("wp", [E, E], bf16, kind="ExternalInput").ap()
    out = nc.dram_tensor("out", [TPC, E], f32, kind="ExternalOutput").ap()

    ev_i = 0

    def evict(dst, src):
        # spread PSUM->SBUF eviction copies across DVE and ACT
        nonlocal ev_i
        if ev_i % 2 == 0:
            nc.vector.tensor_copy(dst, src)
        else:
            nc.scalar.copy(dst, src)
        ev_i += 1

    with tile.TileContext(nc) as tc:
        with (
            tc.tile_pool(name="xp", bufs=1) as xp,
            tc.tile_pool(name="wkvp", bufs=1) as wkvp,
            tc.tile_pool(name="kvp", bufs=1) as kvp,
            tc.tile_pool(name="wqp", bufs=1) as wqp,
            tc.tile_pool(name="wpp", bufs=1) as wpp,
            tc.tile_pool(name="qp", bufs=1) as qp,
            tc.tile_pool(name="mres", bufs=1) as mres,
            tc.tile_pool(name="op", bufs=3) as op,
            tc.tile_pool(name="dram", bufs=1, space="DRAM") as dram,
            tc.tile_pool(name="psA", bufs=4, space="PSUM") as psA,
            tc.tile_pool(name="psM", bufs=2, space="PSUM") as psM,
            tc.tile_pool(name="psB", bufs=2, space="PSUM") as psB,
        ):
            # ---- PE warm-up scratch (memset is the first gpsimd work) ----
            wup = xp.tile([128, 512], bf16, tag="wup", name="wup")
            nc.gpsimd.memset(wup[:].bitcast(f32), 0.0)

            # ---- PE warm-up: ~14 matmuls on zeroed scratch SBUF so the PE
            # p-state is fully ramped (2.4GHz needs 3us of continuous
            # execution) by the time the first real tiles land
            wups = psA.tile([128, 512], f32, tag="big", name="wups")
            for i in range(14):
                nc.tensor.matmul(wups[:], wup[:, 0:128], wup[:],
                                 start=(i == 0), stop=(i == 13))

            # ---- input DMA triggers, spread across queues ----
            # sync: x2 + wA odds interleaved (the kv-A critical stream), wB
            # scalar: wA evens          vector: wq          gpsimd: wp
            x2sb = [xp.tile([128, 1024], bf16, tag=f"x{i}", name=f"x{i}")
                    for i in range(4)]
            wA = [wkvp.tile([128, 1024], bf16, tag=f"wA{e}", name=f"wA{e}")
                  for e in range(8)]
            wB = [wkvp.tile([128, 1024], bf16, tag=f"wB{e}", name=f"wB{e}")
                  for e in range(8)]
            wqsb = [wqp.tile([128, 1024], bf16, tag=f"wq{e}", name=f"wq{e}")
                    for e in range(8)]
            wpsb = [wpp.tile([128, 1024], bf16, tag=f"wp{e}", name=f"wp{e}")
                    for e in range(8)]
            for i in range(4):
                nc.sync.dma_start(x2sb[i][:], x2[:, 1024 * i:1024 * (i + 1)])
                nc.sync.dma_start(wA[2 * i + 1][:],
                                  wkv[128 * (2 * i + 1):128 * (2 * i + 2), 0:1024])
                nc.scalar.dma_start(wA[2 * i][:],
                                    wkv[128 * (2 * i):128 * (2 * i + 1), 0:1024])
            for e in range(8):
                nc.sync.dma_start(wB[e][:], wkv[128 * e:128 * (e + 1), 1024:2048])
                nc.scalar.dma_start(wqsb[e][:], wq[128 * e:128 * (e + 1), :])
                nc.gpsimd.dma_start(wpsb[e][:], wp[128 * e:128 * (e + 1), :])

            def xblk(e, tt):
                # stationary (128, 128) x block: e-features x token slice tt
                return x2sb[e // 2][:, 512 * (e % 2) + 128 * tt:
                                    512 * (e % 2) + 128 * (tt + 1)]

            # kv activations, cols [k0 | v0 | k1 | v1] like wkv
            kvsb = [kvp.tile([128, 2048], bf16, tag=f"kv{tt}", name=f"kv{tt}")
                    for tt in range(4)]
            Mbd = mres.tile([128, 1024], bf16, tag="Mbd")
            nc.gpsimd.memset(Mbd[:].bitcast(f32), 0.0)

            # ---- kv half A, e-outer across all 8 PSUM banks: the PE gets 8
            # matmuls per arriving (x2, wA) pair -- no starvation gaps, and the
            # p-state stays at 2.4GHz behind the warm-up
            psk = [psA.tile([128, 512], f32, tag="big", name=f"psk{tt}")
                   for tt in range(4)]
            psv = ([psM.tile([128, 512], f32, tag="mp", name=f"psv{tt}")
                    for tt in range(2)] +
                   [psB.tile([128, 512], f32, tag="pb", name=f"psv{tt}")
                    for tt in (2, 3)])
            for e in range(8):
                for tt in range(4):
                    nc.tensor.matmul(psk[tt][:], xblk(e, tt), wA[e][:, 0:512],
                                     start=(e == 0), stop=(e == 7))
                for tt in range(4):
                    nc.tensor.matmul(psv[tt][:], xblk(e, tt), wA[e][:, 512:1024],
                                     start=(e == 0), stop=(e == 7))
            for tt in range(4):
                evict(kvsb[tt][:, 0:512], psk[tt][:])
                evict(kvsb[tt][:, 512:1024], psv[tt][:])

            Mr = [None, None]

            def m_half(g):
                # M blocks 4g..4g+3 (diagonal 64x64 sub-blocks) + AllGather
                mp = psM.tile([128, 512], f32, tag="mp", name=f"mp{g}")
                for j in range(4):
                    kcol = 1024 * g + 128 * j
                    vcol = 1024 * g + 512 + 128 * j
                    for tt in range(4):
                        nc.tensor.matmul(
                            mp[:, 128 * j:128 * (j + 1)],
                            kvsb[tt][:, kcol:kcol + 128],
                            kvsb[tt][:, vcol:vcol + 128],
                            start=(tt == 0), stop=(tt == 3),
                        )
                Msb = mres.tile([128, 256], f32, tag=f"Msb{g}", name=f"Msb{g}")
                for j in range(4):
                    nc.vector.tensor_copy(Msb[0:64, 64 * j:64 * j + 64],
                                          mp[0:64, 128 * j:128 * j + 64])
                    nc.vector.tensor_copy(Msb[64:128, 64 * j:64 * j + 64],
                                          mp[64:128, 128 * j + 64:128 * (j + 1)])
                bin_ = dram.tile([128, 256], f32, name=f"bin{g}")
                bo = dram.tile([256, 256], f32, name=f"bout{g}")
                nc.gpsimd.dma_start(bin_[:], Msb[:])
                nc.gpsimd.collective_compute(
                    "AllGather", mybir.AluOpType.bypass, replica_groups=GROUPS,
                    ins=[bin_.opt()], outs=[bo.opt()],
                )
                mrA = mres.tile([128, 256], f32, tag=f"MrA{g}", name=f"MrA{g}")
                mrB = mres.tile([128, 256], f32, tag=f"MrB{g}", name=f"MrB{g}")
                nc.sync.dma_start(mrA[:], bo[0:128, :])   # sync is idle by now
                nc.sync.dma_start(mrB[:], bo[128:256, :])
                Mr[g] = (mrA, mrB)

            def m_post(g):
                # peer-add both ranks' partials into the Mbd diagonal (bf16);
                # split across DVE and Pool so Mbd is ready in half the time
                mrA, mrB = Mr[g]
                for j in range(4):
                    blk = 4 * g + j
                    eng = nc.vector if j % 2 else nc.gpsimd
                    eng.tensor_add(
                        Mbd[0:64, 128 * blk:128 * blk + 64],
                        mrA[0:64, 64 * j:64 * j + 64],
                        mrB[0:64, 64 * j:64 * j + 64])
                    eng.tensor_add(
                        Mbd[64:128, 128 * blk + 64:128 * (blk + 1)],
                        mrA[64:128, 64 * j:64 * j + 64],
                        mrB[64:128, 64 * j:64 * j + 64])

            m_half(0)                     # AllReduce #1 in flight

            # ---- kv half B ----
            for col, base in ((0, 1024), (512, 1536)):      # k1, v1
                for tt in range(4):
                    ps = psA.tile([128, 512], f32, tag="big", name=f"pskvB{base}_{tt}")
                    for e in range(8):
                        nc.tensor.matmul(ps[:], xblk(e, tt), wB[e][:, col:col + 512],
                                         start=(e == 0), stop=(e == 7))
                    evict(kvsb[tt][:, base:base + 512], ps[:])
            m_half(1)                     # AllReduce #2 in flight

            # ---- q blocks 0-3 (feature-major (512f, 512t)) ----
            qsb = [qp.tile([128, TPC], bf16, tag=f"q{f}", name=f"q{f}")
                   for f in range(8)]

            def q_block(fq, pool):
                ps = pool.tile([128, 512], f32, tag=("big" if pool is psA else "mp"),
                               name=f"psq{fq}")
                for e in range(8):
                    nc.tensor.matmul(
                        ps[:],
                        wqsb[e][:, 128 * fq:128 * (fq + 1)],
                        x2sb[e // 2][:, 512 * (e % 2):512 * (e % 2 + 1)],
                        start=(e == 0), stop=(e == 7),
                    )
                evict(qsb[fq][:], ps[:])

            def att_block(blk):
                ps = psM.tile([128, 512], f32, tag="mp", name=f"psatt{blk}")
                nc.tensor.matmul(ps[:], Mbd[:, 128 * blk:128 * (blk + 1)],
                                 qsb[blk][:], start=True, stop=True)
                evict(qsb[blk][:], ps[:])

            for fq in range(8):
                q_block(fq, psM)
            m_post(0)
            for blk in range(4):
                att_block(blk)

            # ---- out waves (tt 0-2), stage 1: att features 0-3 only ----
            # leaves 6 PSUM banks open; stage 2 closes them after att 4-7
            wave1 = []
            for tt in range(2):
                for oc in range(2):
                    ps = psA.tile([128, 512], f32, tag="big", name=f"pso{tt}_{oc}")
                    for f in range(4):
                        nc.tensor.matmul(
                            ps[:],
                            qsb[f][:, 128 * tt:128 * (tt + 1)],
                            wpsb[f][:, 512 * oc:512 * (oc + 1)],
                            start=(f == 0), stop=False,
                        )
                    wave1.append((tt, oc, ps))
            for oc in range(2):
                ps = psB.tile([128, 512], f32, tag="pb", name=f"pso2_{oc}")
                for f in range(4):
                    nc.tensor.matmul(
                        ps[:],
                        qsb[f][:, 256:384],
                        wpsb[f][:, 512 * oc:512 * (oc + 1)],
                        start=(f == 0), stop=False,
                    )
                wave1.append((2, oc, ps))

            m_post(1)
            for blk in range(4, 8):
                att_block(blk)

            st_i = 0

            def store(tt, oc, ps):
                nonlocal st_i
                ot = op.tile([128, 512], f32, tag="osb", name=f"ot{tt}_{oc}")
                evict(ot[:], ps[:])
                eng = nc.sync if st_i % 2 else nc.gpsimd
                st_i += 1
                eng.dma_start(out[128 * tt:128 * (tt + 1),
                                  512 * oc:512 * (oc + 1)], ot[:])

            for tt, oc, ps in wave1:      # stage 2: att features 4-7, close
                for f in range(4, 8):
                    nc.tensor.matmul(
                        ps[:],
                        qsb[f][:, 128 * tt:128 * (tt + 1)],
                        wpsb[f][:, 512 * oc:512 * (oc + 1)],
                        start=False, stop=(f == 7),
                    )
                store(tt, oc, ps)

            for tt in (3,):               # last wave: full accumulation
                for oc in range(2):
                    ps = psA.tile([128, 512], f32, tag="big", name=f"pso{tt}_{oc}")
                    for f in range(8):
                        nc.tensor.matmul(
                            ps[:],
                            qsb[f][:, 128 * tt:128 * (tt + 1)],
                            wpsb[f][:, 512 * oc:512 * (oc + 1)],
                            start=(f == 0), stop=(f == 7),
                        )
                    store(tt, oc, ps)

    nc.compile()
    _built = nc
    return nc


LAST_RESULTS = None  # BassKernelResults of the most recent kernel() call


def kernel(x: np.ndarray, W_qkv: np.ndarray, W_proj: np.ndarray) -> np.ndarray:
    global LAST_RESULTS
    import ml_dtypes
    from concourse import bass_utils

    bf16 = ml_dtypes.bfloat16
    nc = _build()

    x = np.ascontiguousarray(x, dtype=np.float32)
    W_qkv = np.ascontiguousarray(W_qkv, dtype=np.float32)
    W_proj = np.ascontiguousarray(W_proj, dtype=np.float32)

    # head-grouping permutation: grouped feature h*64+j <- original row j*16+h
    perm = np.arange(E).reshape(HD, NH).T.ravel()
    qT = W_qkv[perm].T                                     # (E_in, E_qf)
    kT = (W_qkv[E + perm] * np.float32(HD ** -0.5)).T      # exact scale: 1/8
    vT = W_qkv[2 * E + perm].T
    wkv_np = np.concatenate(
        [kT[:, :512], vT[:, :512], kT[:, 512:], vT[:, 512:]], axis=1
    ).astype(bf16)
    wq_np = qT.astype(bf16)
    wp_np = W_proj.T.astype(bf16)

    in_maps = []
    for c in range(N_CORES):
        b, half = c // 2, c % 2
        xT_c = x[b, half * TPC:(half + 1) * TPC, :].T      # (E, 512)
        # pack e-blocks side by side for 2KB DMA lines:
        # x2[p, 512*e + t] = xT_c[128*e + p, t]
        x2_c = np.ascontiguousarray(
            xT_c.reshape(8, 128, TPC).transpose(1, 0, 2).reshape(128, 4096)
        ).astype(bf16)
        in_maps.append({"x2": x2_c, "wkv": wkv_np, "wq": wq_np, "wp": wp_np})

    import os as _os
    _tc = _os.environ.get("KERNEL_TRACE_CORES")
    _kw = {"trace_cores": [int(v) for v in _tc.split(",")]} if _tc else {}
    res = bass_utils.run_bass_kernel_spmd(nc, in_maps, core_ids=list(range(N_CORES)), **_kw)
    LAST_RESULTS = res

    out = np.empty((B, T, E), dtype=np.float32)
    for c in range(N_CORES):
        b, half = c // 2, c % 2
        out[b, half * TPC:(half + 1) * TPC, :] = res.results[c]["out"]
    return out
